# revision 1
# baseline (speedup 1.0000x reference)
"""DDiT block (adaLN + masked attention + MLP) on 8 TRN2 NeuronCores.

Sharding: data-parallel over (batch, seq): core c handles tokens
[ (c%4)*1024, (c%4+1)*1024 ) of batch c//4.  K/V are AllGathered within
each batch group of 4 cores.  All matmuls run in bf16 with f32 PSUM
accumulation; layernorms, softmax statistics and residuals stay f32.
"""

import sys
from contextlib import ExitStack

sys.path.insert(0, "/opt/trn_rl_repo")

import numpy as np
import ml_dtypes

import concourse.bass as bass
import concourse.mybir as mybir
import concourse.tile as tile
import concourse.bass_utils as _bu
import concourse.bass2jax as _b2j
from concourse.bass_utils import run_bass_kernel_spmd
from concourse.masks import make_identity

# ---------------------------------------------------------------------------
# Patch: this container's walrus accepts at most ONE sync-wait per
# instruction, but Tile emits multi-wait instructions.  Split the extras
# into preceding NoOps on the same engine (engines execute their streams
# in-order, so blocking at a preceding NoOp is semantically identical).
# ---------------------------------------------------------------------------
import json as _json

_orig_compile_bir_kernel = _bu.compile_bir_kernel


def _split_multi_waits_json(bir_json):
    j = _json.loads(bir_json)
    for fn in j.get("functions", []):
        for bb in fn.get("blocks", []):
            new_insts = []
            for ins in bb.get("instructions", []):
                si = ins.get("sync_info")
                if si and si.get("on_wait") and len(si["on_wait"]) > 1:
                    waits = si["on_wait"]
                    for k, w in enumerate(waits[:-1]):
                        new_insts.append(
                            {
                                "debug": ins.get("debug"),
                                "engine": ins["engine"],
                                "ins": [],
                                "outs": [],
                                "name": f"{ins['name']}-w{k}",
                                "opcode": "NoOp",
                                "sync_info": {"on_update": [], "on_wait": [w]},
                            }
                        )
                    si["on_wait"] = [waits[-1]]
                new_insts.append(ins)
            bb["instructions"] = new_insts
    return _json.dumps(j).encode()


def _patched_compile_bir_kernel(bir_json, tmpdir, neff_name="file.neff"):
    return _orig_compile_bir_kernel(_split_multi_waits_json(bir_json), tmpdir, neff_name)


_bu.compile_bir_kernel = _patched_compile_bir_kernel
_b2j.compile_bir_kernel = _patched_compile_bir_kernel

# ---------------------------------------------------------------------------

BF = mybir.dt.bfloat16
F32 = mybir.dt.float32
AF = mybir.ActivationFunctionType
OP = mybir.AluOpType
NPBF = ml_dtypes.bfloat16

N_CORES = 8
B, S, D, H, HD = 2, 4096, 768, 12, 64
TPC = (B * S) // N_CORES  # 1024 tokens per core
NTT = TPC // 128  # 8 token tiles per core
NKT = S // 128  # 32 key tiles per batch
DC = D // 128  # 6 dim chunks
MLPD = 4 * D  # 3072
MC = MLPD // 128  # 24 mlp chunks
LN_EPS = 1e-5
AGR = 1920  # per-rank rows in the all-gather payload


def _mm(nc, out, lhsT, rhs, start, stop, tp=None):
    nc.tensor.matmul(out, lhsT, rhs, start=start, stop=stop, tile_position=tp)


def build_program():
    nc = bass.Bass("TRN2", target_bir_lowering=False, debug=False, num_devices=N_CORES)

    di = {}

    def din(name, shape, dtype):
        di[name] = nc.dram_tensor(name, shape, dtype, kind="ExternalInput")
        return di[name]

    x_sh = din("x_sh", [TPC, D], F32)
    c_col = din("c_col", [128, DC], F32)
    mq_d = din("mq", [128, NTT], F32)
    mbias_d = din("mbias", [128, NKT], F32)
    wqk_d = din("wqkT", [D, 2 * D], BF)
    wqkp_d = din("wqkTp", [D, 2 * D], BF)
    wv_d = din("wvT", [D, D], BF)
    wout_d = din("woutT", [D, D], BF)
    w1_d = din("w1T", [D, MLPD], BF)
    w2_d = din("w2T", [MLPD, D], BF)
    wadam_d = din("wadam", [D, 4 * D], BF)
    wadag_d = din("wadag", [D, 2 * D], BF)
    badam_d = din("badam", [128, 4 * DC], F32)
    badag_d = din("badag", [1, 2 * D], F32)
    b1_d = din("b1c", [128, MC], F32)
    b2_d = din("b2r", [1, D], F32)
    ln1_d = din("ln1c", [128, DC], F32)
    ln2_d = din("ln2c", [128, DC], F32)
    cosT2_d = din("cosT2", [128, TPC], BF)
    sinT2_d = din("sinT2s", [128, TPC], BF)
    cosV_d = din("cosV", [TPC, D], BF)
    sinV_d = din("sinVs", [TPC, D], BF)

    out_sh = nc.dram_tensor("out_sh", [TPC, D], F32, kind="ExternalOutput")

    def chunked(dram_ap, nchunk):
        # [nchunk*128, F] dram -> [128, nchunk, F] view for SBUF load
        return dram_ap[:].rearrange("(c p) f -> p c f", p=128)

    with tile.TileContext(nc) as tc, ExitStack() as ctx:
        const = ctx.enter_context(tc.tile_pool(name="const", bufs=1))
        # ---- constants ----
        ident = const.tile([128, 128], BF)
        make_identity(nc, ident)
        ones_col = const.tile([128, 1], BF)
        nc.vector.memset(ones_col, 1.0)
        ones_row = const.tile([1, 128], BF)
        nc.vector.memset(ones_row, 1.0)
        eps_t = const.tile([128, 1], F32)
        nc.vector.memset(eps_t, LN_EPS)
        mq_sb = const.tile([128, NTT], F32)
        nc.sync.dma_start(out=mq_sb, in_=mq_d[:])
        mq1m = const.tile([128, NTT], F32)
        nc.vector.tensor_scalar(mq1m, mq_sb, -1.0, 1.0, op0=OP.mult, op1=OP.add)
        mbias_sb = const.tile([128, NKT], F32)
        nc.sync.dma_start(out=mbias_sb, in_=mbias_d[:])
        c_sb = const.tile([128, DC], F32)
        nc.sync.dma_start(out=c_sb, in_=c_col[:])
        c_bf = const.tile([128, DC], BF)
        nc.vector.tensor_copy(c_bf, c_sb)
        cosT2 = const.tile([128, TPC], BF)
        nc.sync.dma_start(out=cosT2, in_=cosT2_d[:])
        sinT2 = const.tile([128, TPC], BF)
        nc.sync.dma_start(out=sinT2, in_=sinT2_d[:])
        ln1c = const.tile([128, DC], F32)
        nc.sync.dma_start(out=ln1c, in_=ln1_d[:])
        ln2c = const.tile([128, DC], F32)
        nc.sync.dma_start(out=ln2c, in_=ln2_d[:])
        badam_sb = const.tile([128, 4 * DC], F32)
        nc.sync.dma_start(out=badam_sb, in_=badam_d[:])
        badag_sb = const.tile([1, 2 * D], F32)
        nc.sync.dma_start(out=badag_sb, in_=badag_d[:])
        b1c_sb = const.tile([128, MC], F32)
        nc.sync.dma_start(out=b1c_sb, in_=b1_d[:])
        b2r_sb = const.tile([1, D], F32)
        nc.sync.dma_start(out=b2r_sb, in_=b2_d[:])

        modm = const.tile([128, 4 * DC], F32)  # shift_msa|scale_msa|shift_mlp|scale_mlp
        gates_row = const.tile([1, 2 * D], F32)  # gate_msa | gate_mlp
        A1 = const.tile([128, DC], F32)
        A2 = const.tile([128, DC], F32)
        B1bf = const.tile([128, DC], BF)
        B2bf = const.tile([128, DC], BF)
        gateB = const.tile([128, D], F32)
        gmlB = const.tile([128, D], F32)
        gyuB = const.tile([128, D], F32)
        gb2B = const.tile([128, D], F32)
        bq_sb = const.tile([128, 2 * DC], F32)
        bqp_sb = const.tile([128, 2 * DC], F32)
        bv_bf = const.tile([1, D], BF)

        # dram staging for the all-gather
        dpool = ctx.enter_context(tc.tile_pool(name="dram", bufs=1, space="DRAM"))
        agin = dpool.tile([AGR, 1024], BF)
        agout = dpool.tile([4 * AGR, 1024], BF)
        bc_dram = dpool.tile([4, D], F32)

        def bcast_row(out_tile, row_ap, slot):
            nc.sync.dma_start(out=bc_dram[slot : slot + 1, :], in_=row_ap)
            nc.sync.dma_start(
                out=out_tile,
                in_=bass.AP(
                    tensor=bc_dram.tensor,
                    offset=bc_dram.offset + slot * D,
                    ap=[[0, 128], [1, D]],
                ),
            )

        # ---- adaLN modulation ----
        with (
            tc.tile_pool(name="ada", bufs=1) as ada,
            tc.tile_pool(name="psA", bufs=1, space="PSUM") as psA,
        ):
            wadam_sb = ada.tile([128, DC, 4 * D], BF)
            nc.sync.dma_start(out=wadam_sb, in_=chunked(wadam_d, DC))
            wadag_sb = ada.tile([128, DC, 2 * D], BF)
            nc.sync.dma_start(out=wadag_sb, in_=chunked(wadag_d, DC))

            mod_ps = psA.tile([128, 4 * DC], F32)
            for jc in range(4 * DC):
                for ic in range(DC):
                    _mm(nc, mod_ps[:, jc : jc + 1],
                        wadam_sb[:, ic, jc * 128 : (jc + 1) * 128],
                        c_bf[:, ic : ic + 1], ic == 0, ic == DC - 1)
            nc.vector.tensor_add(modm, mod_ps, badam_sb)

            for nch in range(3):
                gps = psA.tile([1, 512], F32, tag="gps")
                for ic in range(DC):
                    _mm(nc, gps, c_bf[:, ic : ic + 1],
                        wadag_sb[:, ic, nch * 512 : (nch + 1) * 512], ic == 0, ic == DC - 1)
                nc.vector.tensor_add(
                    gates_row[:, nch * 512 : (nch + 1) * 512], gps,
                    badag_sb[:, nch * 512 : (nch + 1) * 512])

            # A1 = ln1w*(1+scale_msa); A2 = ln2w*(1+scale_mlp)
            tmp1 = ada.tile([128, DC], F32)
            nc.vector.tensor_scalar_add(tmp1, modm[:, DC : 2 * DC], 1.0)
            nc.vector.tensor_mul(A1, tmp1, ln1c)
            tmp2 = ada.tile([128, DC], F32)
            nc.vector.tensor_scalar_add(tmp2, modm[:, 3 * DC : 4 * DC], 1.0)
            nc.vector.tensor_mul(A2, tmp2, ln2c)
            nc.vector.tensor_copy(B1bf, modm[:, 0:DC])
            nc.vector.tensor_copy(B2bf, modm[:, 2 * DC : 3 * DC])

            bcast_row(gateB, gates_row[0:1, 0:D], 0)
            bcast_row(gmlB, gates_row[0:1, D : 2 * D], 1)
            r2row = ada.tile([1, D], F32)
            nc.vector.tensor_mul(r2row, gates_row[0:1, D : 2 * D], b2r_sb)
            bcast_row(gb2B, r2row, 2)

        # persistent activation pools (LIFO: x2p -> outp -> qp -> inner phases)
        x2p = ctx.enter_context(tc.tile_pool(name="x2p", bufs=1))
        x2_sb = x2p.tile([128, NTT, D], F32)
        outp_cm = tc.tile_pool(name="outp", bufs=1)
        outp = outp_cm.__enter__()
        oTall = outp.tile([128, DC, TPC], BF)
        woutT_sb = outp.tile([128, DC, D], BF)
        mv_bf = outp.tile([128, DC], BF)
        yu_row = outp.tile([1, D], F32)
        qp_cm = tc.tile_pool(name="qp", bufs=1)
        qp = qp_cm.__enter__()
        qT_sb = qp.tile([128, DC, TPC], BF)

        # ---- phase B: LN1, transposes, qkv + rope, AG staging ----
        with (
            tc.tile_pool(name="bpool", bufs=1) as bp,
            tc.tile_pool(name="wqk", bufs=1) as wq,
            tc.tile_pool(name="lnt", bufs=3) as lnt,
            tc.tile_pool(name="ropep", bufs=3) as rp,
        ):
            xhatT = bp.tile([128, DC, TPC], BF)
            kTloc = bp.tile([128, DC, TPC], BF)
            vloc = bp.tile([128, NTT, D], BF)

            wqk_sb = wq.tile([128, DC, 2 * D], BF)
            nc.sync.dma_start(out=wqk_sb, in_=chunked(wqk_d, DC))
            wqkp_sb = wq.tile([128, DC, 2 * D], BF)
            nc.sync.dma_start(out=wqkp_sb, in_=chunked(wqkp_d, DC))
            wv_sb = wq.tile([128, DC, D], BF)
            nc.sync.dma_start(out=wv_sb, in_=chunked(wv_d, DC))

            # LN1 + PE transposes
            psT_cm = tc.tile_pool(name="psT", bufs=2, space="PSUM")
            psT = psT_cm.__enter__()
            for tt in range(NTT):
                x_t = lnt.tile([128, D], F32, tag="x", bufs=2)
                nc.sync.dma_start(out=x_t, in_=x_sh[tt * 128 : (tt + 1) * 128, :])
                stats = lnt.tile([128, 3, 6], F32, tag="st")
                for sg in range(3):
                    nc.vector.bn_stats(out=stats[:, sg, :], in_=x_t[:, sg * 256 : (sg + 1) * 256])
                mv = lnt.tile([128, 2], F32, tag="mv")
                nc.vector.bn_aggr(out=mv, in_=stats)
                sq = lnt.tile([128, 1], F32, tag="sq")
                nc.scalar.activation(out=sq, in_=mv[:, 1:2], func=AF.Sqrt, bias=eps_t, scale=1.0)
                rstd = lnt.tile([128, 1], F32, tag="rs")
                nc.vector.reciprocal(out=rstd, in_=sq)
                nmr = lnt.tile([128, 1], F32, tag="nm")
                nc.vector.scalar_tensor_tensor(
                    out=nmr, in0=mv[:, 0:1], scalar=-1.0, in1=rstd, op0=OP.mult, op1=OP.mult)
                xhat_bf = lnt.tile([128, D], BF, tag="xh")
                nc.vector.tensor_scalar(xhat_bf, x_t, rstd, nmr, op0=OP.mult, op1=OP.add)
                for dc in range(DC):
                    pt = psT.tile([128, 128], BF, tag="pt")
                    nc.tensor.transpose(pt, xhat_bf[:, dc * 128 : (dc + 1) * 128], ident)
                    nc.vector.tensor_copy(
                        out=xhatT[:, dc, tt * 128 : (tt + 1) * 128], in_=pt)

            psT_cm.__exit__(None, None, None)

            # bias columns/rows from UNSCALED weights
            psBias_cm = tc.tile_pool(name="psBias", bufs=1, space="PSUM")
            psQ = psBias_cm.__enter__()
            psV = psQ
            pbq = psQ.tile([128, 2 * DC], F32, tag="pbq")
            pbqp = psQ.tile([128, 2 * DC], F32, tag="pbqp")
            for fc in range(2 * DC):
                for ic in range(DC):
                    _mm(nc, pbq[:, fc : fc + 1],
                        wqk_sb[:, ic, fc * 128 : (fc + 1) * 128],
                        B1bf[:, ic : ic + 1], ic == 0, ic == DC - 1)
                for ic in range(DC):
                    _mm(nc, pbqp[:, fc : fc + 1],
                        wqkp_sb[:, ic, fc * 128 : (fc + 1) * 128],
                        B1bf[:, ic : ic + 1], ic == 0, ic == DC - 1)
            nc.vector.tensor_copy(bq_sb, pbq)
            nc.vector.tensor_copy(bqp_sb, pbqp)
            bv_ps1 = psV.tile([1, 512], F32, tag="bv")
            bv_ps2 = psV.tile([1, 256], F32, tag="bv2")
            for ic in range(DC):
                _mm(nc, bv_ps1, B1bf[:, ic : ic + 1], wv_sb[:, ic, 0:512], ic == 0, ic == DC - 1)
                _mm(nc, bv_ps2, B1bf[:, ic : ic + 1], wv_sb[:, ic, 512:768], ic == 0, ic == DC - 1)
            nc.vector.tensor_copy(bv_bf[:, 0:512], bv_ps1)
            nc.vector.tensor_copy(bv_bf[:, 512:768], bv_ps2)

            psBias_cm.__exit__(None, None, None)

            # scale weights in place by A1 (per dim-chunk)
            for ic in range(DC):
                nc.vector.tensor_scalar_mul(wqk_sb[:, ic, :], wqk_sb[:, ic, :], A1[:, ic : ic + 1])
                nc.vector.tensor_scalar_mul(wqkp_sb[:, ic, :], wqkp_sb[:, ic, :], A1[:, ic : ic + 1])
                nc.vector.tensor_scalar_mul(wv_sb[:, ic, :], wv_sb[:, ic, :], A1[:, ic : ic + 1])

            # k first (feeds the AG), then q
            psQK_cm = tc.tile_pool(name="psQK", bufs=2, space="PSUM")
            psQ = psQK_cm.__enter__()
            for fc in list(range(DC, 2 * DC)) + list(range(DC)):
                dest = kTloc if fc >= DC else qT_sb
                fcd = fc % DC
                for q2 in range(2):
                    pa = psQ.tile([128, 512], F32, tag="pa")
                    pb = psQ.tile([128, 512], F32, tag="pb")
                    for ic in range(DC):
                        _mm(nc, pa, wqk_sb[:, ic, fc * 128 : (fc + 1) * 128],
                            xhatT[:, ic, q2 * 512 : (q2 + 1) * 512], ic == 0, ic == DC - 1)
                    for ic in range(DC):
                        _mm(nc, pb, wqkp_sb[:, ic, fc * 128 : (fc + 1) * 128],
                            xhatT[:, ic, q2 * 512 : (q2 + 1) * 512], ic == 0, ic == DC - 1)
                    t1 = rp.tile([128, 512], BF, tag="t1")
                    nc.vector.scalar_tensor_tensor(
                        out=t1, in0=pa, scalar=bq_sb[:, fc : fc + 1],
                        in1=cosT2[:, q2 * 512 : (q2 + 1) * 512], op0=OP.add, op1=OP.mult)
                    t2 = rp.tile([128, 512], BF, tag="t2")
                    nc.vector.scalar_tensor_tensor(
                        out=t2, in0=pb, scalar=bqp_sb[:, fc : fc + 1],
                        in1=sinT2[:, q2 * 512 : (q2 + 1) * 512], op0=OP.add, op1=OP.mult)
                    nc.vector.tensor_add(
                        dest[:, fcd, q2 * 512 : (q2 + 1) * 512], t1, t2)

            psQK_cm.__exit__(None, None, None)

            # v + rope (+ write AG staging per token tile)
            psVl_cm = tc.tile_pool(name="psVl", bufs=2, space="PSUM")
            psV = psVl_cm.__enter__()
            for tt in range(NTT):
                pv1 = psV.tile([128, 512], F32, tag="pv1")
                pv2 = psV.tile([128, 256], F32, tag="pv2")
                for ic in range(DC):
                    _mm(nc, pv1, xhatT[:, ic, tt * 128 : (tt + 1) * 128],
                        wv_sb[:, ic, 0:512], ic == 0, False)
                _mm(nc, pv1, ones_row, bv_bf[:, 0:512], False, True)
                for ic in range(DC):
                    _mm(nc, pv2, xhatT[:, ic, tt * 128 : (tt + 1) * 128],
                        wv_sb[:, ic, 512:768], ic == 0, False)
                _mm(nc, pv2, ones_row, bv_bf[:, 512:768], False, True)

                cv = rp.tile([128, D], BF, tag="cv", bufs=2)
                nc.sync.dma_start(out=cv, in_=cosV_d[tt * 128 : (tt + 1) * 128, :])
                sv = rp.tile([128, D], BF, tag="sv", bufs=2)
                nc.sync.dma_start(out=sv, in_=sinV_d[tt * 128 : (tt + 1) * 128, :])

                tm1 = rp.tile([128, D], BF, tag="tm1", bufs=2)
                nc.vector.tensor_mul(tm1[:, 0:512], pv1, cv[:, 0:512])
                nc.vector.tensor_mul(tm1[:, 512:768], pv2, cv[:, 512:768])
                tm2 = rp.tile([128, D], BF, tag="tm2", bufs=2)
                # half-swap within each head (8 heads in pv1, 4 in pv2)
                p1v = pv1.rearrange("p (h two j) -> p h two j", two=2, j=32)
                p2v = pv2.rearrange("p (h two j) -> p h two j", two=2, j=32)
                t2v = tm2.rearrange("p (h two j) -> p h two j", two=2, j=32)
                sv_v = sv.rearrange("p (h two j) -> p h two j", two=2, j=32)
                nc.vector.tensor_mul(t2v[:, 0:8, 0, :], p1v[:, :, 1, :], sv_v[:, 0:8, 0, :])
                nc.vector.tensor_mul(t2v[:, 0:8, 1, :], p1v[:, :, 0, :], sv_v[:, 0:8, 1, :])
                nc.vector.tensor_mul(t2v[:, 8:12, 0, :], p2v[:, :, 1, :], sv_v[:, 8:12, 0, :])
                nc.vector.tensor_mul(t2v[:, 8:12, 1, :], p2v[:, :, 0, :], sv_v[:, 8:12, 1, :])
                nc.vector.tensor_add(vloc[:, tt, :], tm1, tm2)

            # meanV partial sums over this core's tokens
            pmv = psV.tile([128, DC], F32, tag="pmv")
            for dc in range(DC):
                for tt in range(NTT):
                    _mm(nc, pmv[:, dc : dc + 1],
                        vloc[:, tt, dc * 128 : (dc + 1) * 128], ones_col,
                        tt == 0, tt == NTT - 1)
            mvp_bf = bp.tile([128, DC], BF)
            nc.vector.tensor_copy(mvp_bf, pmv)
            psVl_cm.__exit__(None, None, None)

            # stage the AG payload
            for fc in range(DC):
                nc.sync.dma_start(out=agin[fc * 128 : (fc + 1) * 128, :], in_=kTloc[:, fc, :])
            for tt in range(NTT):
                nc.sync.dma_start(
                    out=agin[D + tt * 128 : D + (tt + 1) * 128, 0:D], in_=vloc[:, tt, :])
            nc.sync.dma_start(out=agin[1792:1920, 0:DC], in_=mvp_bf)

            nc.gpsimd.collective_compute(
                "AllGather",
                OP.bypass,
                replica_groups=[[0, 1, 2, 3], [4, 5, 6, 7]],
                ins=[agin.opt()],
                outs=[agout.opt()],
            )

        # ---- attention ----
        with (
            tc.tile_pool(name="kv", bufs=1) as kv,
            tc.tile_pool(name="wp", bufs=3) as wp,
            tc.tile_pool(name="sp", bufs=1) as sp,
            tc.tile_pool(name="psSc", bufs=2, space="PSUM") as psSc,
            tc.tile_pool(name="psO", bufs=1, space="PSUM") as psO,
            tc.tile_pool(name="psS", bufs=1, space="PSUM") as psS,
        ):
            KTf = kv.tile([128, DC, S], BF)
            Vf = kv.tile([128, NKT, D], BF)
            for r in range(4):
                for fc in range(DC):
                    nc.sync.dma_start(
                        out=KTf[:, fc, r * 1024 : (r + 1) * 1024],
                        in_=agout[r * AGR + fc * 128 : r * AGR + (fc + 1) * 128, :])
                for tt in range(NTT):
                    nc.sync.dma_start(
                        out=Vf[:, r * NTT + tt, :],
                        in_=agout[r * AGR + D + tt * 128 : r * AGR + D + (tt + 1) * 128, 0:D])
            nc.sync.dma_start(out=woutT_sb, in_=chunked(wout_d, DC))

            # meanV: sum of 4 rank partials, then /S
            mvps = [kv.tile([128, DC], BF, tag=f"mvp{r}", name=f"mvp{r}") for r in range(4)]
            for r in range(4):
                nc.sync.dma_start(
                    out=mvps[r], in_=agout[r * AGR + 1792 : r * AGR + 1920, 0:DC])
            mvf1 = kv.tile([128, DC], F32)
            nc.vector.tensor_add(mvf1, mvps[0], mvps[1])
            mvf2 = kv.tile([128, DC], F32)
            nc.vector.tensor_add(mvf2, mvps[2], mvps[3])
            mvf = kv.tile([128, DC], F32)
            nc.vector.tensor_add(mvf, mvf1, mvf2)
            nc.vector.tensor_scalar_mul(mv_bf, mvf, 1.0 / S)

            # y_unif = Wout @ meanV   (row form), then gyuB broadcast
            yps1 = psS.tile([1, 512], F32, tag="yu1")
            yps2 = psS.tile([1, 256], F32, tag="yu2")
            for ic in range(DC):
                _mm(nc, yps1, mv_bf[:, ic : ic + 1], woutT_sb[:, ic, 0:512], ic == 0, ic == DC - 1)
                _mm(nc, yps2, mv_bf[:, ic : ic + 1], woutT_sb[:, ic, 512:768], ic == 0, ic == DC - 1)
            nc.vector.tensor_copy(yu_row[:, 0:512], yps1)
            nc.vector.tensor_copy(yu_row[:, 512:768], yps2)
            gyu_row = kv.tile([1, D], F32)
            nc.vector.tensor_mul(gyu_row, yu_row, gates_row[0:1, 0:D])
            bcast_row(gyuB, gyu_row, 3)

            rs_dram = dpool.tile([2, 512], F32)

            for hp in range(DC):
                for q2 in range(2):
                    oT_ps = psO.tile([128, 512], F32, tag="ot")
                    s_ps = psS.tile([128, 512], F32, tag="sps")
                    for kt in range(NKT):
                        sc = psSc.tile([128, 1024], F32, tag="sc")
                        _mm(nc, sc[:, 0:512],
                            KTf[0:64, hp, kt * 128 : (kt + 1) * 128],
                            qT_sb[0:64, hp, q2 * 512 : (q2 + 1) * 512],
                            True, True, tp=(0, 0))
                        _mm(nc, sc[:, 512:1024],
                            KTf[64:128, hp, kt * 128 : (kt + 1) * 128],
                            qT_sb[64:128, hp, q2 * 512 : (q2 + 1) * 512],
                            True, True, tp=(64, 0))
                        w = wp.tile([128, 1024], BF, tag="w")
                        nc.scalar.activation(
                            out=w, in_=sc, func=AF.Exp,
                            bias=mbias_sb[:, kt : kt + 1], scale=0.125)
                        _mm(nc, oT_ps[0:64, :],
                            Vf[:, kt, hp * 128 : hp * 128 + 64], w[:, 0:512],
                            kt == 0, kt == NKT - 1, tp=(0, 0))
                        _mm(nc, oT_ps[64:128, :],
                            Vf[:, kt, hp * 128 + 64 : hp * 128 + 128], w[:, 512:1024],
                            kt == 0, kt == NKT - 1, tp=(0, 64))
                        _mm(nc, s_ps[0:1, :], ones_col, w[:, 0:512],
                            kt == 0, kt == NKT - 1, tp=(0, 0))
                        _mm(nc, s_ps[32:33, :], ones_col, w[:, 512:1024],
                            kt == 0, kt == NKT - 1, tp=(0, 32))
                    rsA = sp.tile([1, 512], F32, tag="rsA")
                    nc.vector.reciprocal(out=rsA, in_=s_ps[0:1, :])
                    rsB = sp.tile([1, 512], F32, tag="rsB")
                    nc.vector.reciprocal(out=rsB, in_=s_ps[32:33, :])
                    # broadcast via dram bounce (partition-step-0 read)
                    nc.sync.dma_start(out=rs_dram[0:1, :], in_=rsA)
                    nc.sync.dma_start(out=rs_dram[1:2, :], in_=rsB)
                    rsb = sp.tile([128, 512], F32, tag="rsb")
                    nc.sync.dma_start(
                        out=rsb[0:64, :],
                        in_=bass.AP(tensor=rs_dram.tensor, offset=rs_dram.offset,
                                    ap=[[0, 64], [1, 512]]))
                    nc.sync.dma_start(
                        out=rsb[64:128, :],
                        in_=bass.AP(tensor=rs_dram.tensor, offset=rs_dram.offset + 512,
                                    ap=[[0, 64], [1, 512]]))
                    nc.vector.tensor_mul(
                        oTall[:, hp, q2 * 512 : (q2 + 1) * 512], oT_ps, rsb)

        # ---- out-proj + residual/blend ----
        with (
            tc.tile_pool(name="xop", bufs=3) as xop,
            tc.tile_pool(name="psY", bufs=2, space="PSUM") as psY,
        ):
            for tt in range(NTT):
                y1 = psY.tile([128, 512], F32, tag="y1")
                y2 = psY.tile([128, 256], F32, tag="y2")
                for fc in range(DC):
                    _mm(nc, y1, oTall[:, fc, tt * 128 : (tt + 1) * 128],
                        woutT_sb[:, fc, 0:512], fc == 0, fc == DC - 1)
                for fc in range(DC):
                    _mm(nc, y2, oTall[:, fc, tt * 128 : (tt + 1) * 128],
                        woutT_sb[:, fc, 512:768], fc == 0, fc == DC - 1)
                xs = xop.tile([128, D], F32, tag="xs")
                nc.sync.dma_start(out=xs, in_=x_sh[tt * 128 : (tt + 1) * 128, :])
                t1 = xop.tile([128, D], F32, tag="t1")
                nc.vector.scalar_tensor_tensor(
                    out=t1[:, 0:512], in0=y1, scalar=mq_sb[:, tt : tt + 1],
                    in1=gateB[:, 0:512], op0=OP.mult, op1=OP.mult)
                nc.vector.scalar_tensor_tensor(
                    out=t1[:, 512:768], in0=y2, scalar=mq_sb[:, tt : tt + 1],
                    in1=gateB[:, 512:768], op0=OP.mult, op1=OP.mult)
                t2 = xop.tile([128, D], F32, tag="t2")
                nc.vector.scalar_tensor_tensor(
                    out=t2, in0=gyuB, scalar=mq1m[:, tt : tt + 1], in1=xs,
                    op0=OP.mult, op1=OP.add)
                nc.vector.tensor_add(x2_sb[:, tt, :], t1, t2)

        qp_cm.__exit__(None, None, None)
        outp_cm.__exit__(None, None, None)

        # ---- LN2 + MLP ----
        with (
            tc.tile_pool(name="mlp", bufs=1) as mp,
            tc.tile_pool(name="ln2t", bufs=3) as ln2t,
            tc.tile_pool(name="psT2", bufs=1, space="PSUM") as psT2,
            tc.tile_pool(name="psH", bufs=2, space="PSUM") as psH,
            tc.tile_pool(name="psM", bufs=1, space="PSUM") as psM,
        ):
            xhat2T = mp.tile([128, DC, TPC], BF)
            for tt in range(NTT):
                stats = ln2t.tile([128, 3, 6], F32, tag="st")
                for sg in range(3):
                    nc.vector.bn_stats(
                        out=stats[:, sg, :], in_=x2_sb[:, tt, sg * 256 : (sg + 1) * 256])
                mv = ln2t.tile([128, 2], F32, tag="mv")
                nc.vector.bn_aggr(out=mv, in_=stats)
                sq = ln2t.tile([128, 1], F32, tag="sq")
                nc.scalar.activation(out=sq, in_=mv[:, 1:2], func=AF.Sqrt, bias=eps_t, scale=1.0)
                rstd = ln2t.tile([128, 1], F32, tag="rs")
                nc.vector.reciprocal(out=rstd, in_=sq)
                nmr = ln2t.tile([128, 1], F32, tag="nm")
                nc.vector.scalar_tensor_tensor(
                    out=nmr, in0=mv[:, 0:1], scalar=-1.0, in1=rstd, op0=OP.mult, op1=OP.mult)
                xh2 = ln2t.tile([128, D], BF, tag="xh", bufs=2)
                nc.vector.tensor_scalar(xh2, x2_sb[:, tt, :], rstd, nmr, op0=OP.mult, op1=OP.add)
                for dc in range(DC):
                    pt = psT2.tile([128, 128], BF, tag="pt")
                    nc.tensor.transpose(pt, xh2[:, dc * 128 : (dc + 1) * 128], ident)
                    nc.vector.tensor_copy(
                        out=xhat2T[:, dc, tt * 128 : (tt + 1) * 128], in_=pt)

            w1_sb = mp.tile([128, DC, MLPD], BF)
            nc.sync.dma_start(out=w1_sb, in_=chunked(w1_d, DC))
            # b1' from unscaled W1
            pb1 = psH.tile([128, MC], F32, tag="pb1")
            for fc1 in range(MC):
                for ic in range(DC):
                    _mm(nc, pb1[:, fc1 : fc1 + 1],
                        w1_sb[:, ic, fc1 * 128 : (fc1 + 1) * 128],
                        B2bf[:, ic : ic + 1], ic == 0, ic == DC - 1)
            b1full = mp.tile([128, MC], F32)
            nc.vector.tensor_add(b1full, pb1, b1c_sb)
            for ic in range(DC):
                nc.vector.tensor_scalar_mul(w1_sb[:, ic, :], w1_sb[:, ic, :], A2[:, ic : ic + 1])

            g1_sb = mp.tile([128, MC, TPC], BF)
            for fc1 in range(MC):
                for q2 in range(2):
                    hps = psH.tile([128, 512], F32, tag="hps")
                    for ic in range(DC):
                        _mm(nc, hps, w1_sb[:, ic, fc1 * 128 : (fc1 + 1) * 128],
                            xhat2T[:, ic, q2 * 512 : (q2 + 1) * 512], ic == 0, ic == DC - 1)
                    nc.scalar.activation(
                        out=g1_sb[:, fc1, q2 * 512 : (q2 + 1) * 512], in_=hps,
                        func=AF.Gelu_apprx_tanh, bias=b1full[:, fc1 : fc1 + 1], scale=1.0)

            w2_sb = mp.tile([128, MC, D], BF)
            nc.sync.dma_start(out=w2_sb, in_=chunked(w2_d, MC))
            for tt in range(NTT):
                m1 = psM.tile([128, 512], F32, tag="m1")
                m2 = psM.tile([128, 256], F32, tag="m2")
                for fc1 in range(MC):
                    _mm(nc, m1, g1_sb[:, fc1, tt * 128 : (tt + 1) * 128],
                        w2_sb[:, fc1, 0:512], fc1 == 0, fc1 == MC - 1)
                for fc1 in range(MC):
                    _mm(nc, m2, g1_sb[:, fc1, tt * 128 : (tt + 1) * 128],
                        w2_sb[:, fc1, 512:768], fc1 == 0, fc1 == MC - 1)
                u1 = ln2t.tile([128, D], F32, tag="u1", bufs=2)
                nc.vector.tensor_mul(u1[:, 0:512], m1, gmlB[:, 0:512])
                nc.vector.tensor_mul(u1[:, 512:768], m2, gmlB[:, 512:768])
                x3 = ln2t.tile([128, D], F32, tag="x3", bufs=2)
                nc.vector.tensor_add(x3, u1, x2_sb[:, tt, :])
                nc.vector.tensor_add(x3, x3, gb2B)
                nc.sync.dma_start(out=out_sh[tt * 128 : (tt + 1) * 128, :], in_=x3)

    return nc


_PROGRAM = None
_LAST_IN_MAPS = None


def _get_program():
    global _PROGRAM
    if _PROGRAM is None:
        _PROGRAM = build_program()
    return _PROGRAM


def _bf(a):
    return np.ascontiguousarray(np.asarray(a, np.float32)).astype(NPBF)


def kernel(x, cos, sin, c, attention_mask, ln1_w, Wqkv, Wout, ln2_w, W1, b1, W2, b2,
           Wada, bada):
    x = np.asarray(x, np.float32)
    cos2d = np.asarray(cos, np.float32).reshape(S, HD)
    sin2d = np.asarray(sin, np.float32).reshape(S, HD)
    c = np.asarray(c, np.float32)
    m = np.asarray(attention_mask)
    mf = m.astype(np.float32)

    WqkvT = np.asarray(Wqkv, np.float32).T  # [768, 2304]
    f = np.arange(2 * D)
    perm = (f // HD) * HD + (f % HD + HD // 2) % HD
    wqkT = _bf(WqkvT[:, : 2 * D])
    wqkTp = _bf(WqkvT[:, : 2 * D][:, perm])
    wvT = _bf(WqkvT[:, 2 * D :])
    woutT = _bf(np.asarray(Wout, np.float32).T)
    w1T = _bf(np.asarray(W1, np.float32).T)
    w2T = _bf(np.asarray(W2, np.float32).T)
    WadaT = np.asarray(Wada, np.float32).T  # [768, 4608]
    wadam = _bf(np.concatenate(
        [WadaT[:, 0:768], WadaT[:, 768:1536], WadaT[:, 2304:3072], WadaT[:, 3072:3840]],
        axis=1))
    wadag = _bf(np.concatenate([WadaT[:, 1536:2304], WadaT[:, 3840:4608]], axis=1))
    bada = np.asarray(bada, np.float32)
    badam = np.ascontiguousarray(
        np.concatenate([bada[0:768], bada[768:1536], bada[2304:3072], bada[3072:3840]])
        .reshape(4 * DC, 128).T)
    badag = np.ascontiguousarray(
        np.concatenate([bada[1536:2304], bada[3840:4608]])[None, :])
    b1c = np.ascontiguousarray(np.asarray(b1, np.float32).reshape(MC, 128).T)
    b2r = np.ascontiguousarray(np.asarray(b2, np.float32)[None, :])
    ln1c = np.ascontiguousarray(np.asarray(ln1_w, np.float32).reshape(DC, 128).T)
    ln2c = np.ascontiguousarray(np.asarray(ln2_w, np.float32).reshape(DC, 128).T)

    sgn = np.where(np.arange(HD) < HD // 2, -1.0, 1.0).astype(np.float32)
    cosT = cos2d.T  # [64, 4096]
    sinTs = sin2d.T * sgn[:, None]
    cosT2_full = np.tile(cosT, (2, 1))  # [128, 4096]
    sinT2_full = np.tile(sinTs, (2, 1))
    cosV_full = np.tile(cos2d, (1, H))  # [4096, 768]
    sinV_full = np.tile(sin2d * sgn[None, :], (1, H))

    nc = _get_program()

    in_maps = []
    for core in range(N_CORES):
        bi, qp = core // 4, core % 4
        t0, t1 = qp * TPC, (qp + 1) * TPC
        mb_core = np.ascontiguousarray(
            ((mf[bi] - 1.0) * 1e9).reshape(NKT, 128).T)  # [128, 32]
        mq_core = np.ascontiguousarray(mf[bi, t0:t1].reshape(NTT, 128).T)  # [128, 8]
        c_core = np.ascontiguousarray(c[bi].reshape(DC, 128).T)  # [128, 6]
        in_maps.append({
            "x_sh": np.ascontiguousarray(x[bi, t0:t1]),
            "c_col": c_core,
            "mq": mq_core,
            "mbias": mb_core,
            "wqkT": wqkT, "wqkTp": wqkTp, "wvT": wvT, "woutT": woutT,
            "w1T": w1T, "w2T": w2T, "wadam": wadam, "wadag": wadag,
            "badam": badam, "badag": badag, "b1c": b1c, "b2r": b2r,
            "ln1c": ln1c, "ln2c": ln2c,
            "cosT2": _bf(cosT2_full[:, t0:t1]),
            "sinT2s": _bf(sinT2_full[:, t0:t1]),
            "cosV": _bf(cosV_full[t0:t1]),
            "sinVs": _bf(sinV_full[t0:t1]),
        })

    global _LAST_IN_MAPS
    _LAST_IN_MAPS = in_maps
    res = run_bass_kernel_spmd(nc, in_maps, core_ids=list(range(N_CORES)))

    out = np.empty((B, S, D), np.float32)
    for core in range(N_CORES):
        bi, qp = core // 4, core % 4
        out[bi, qp * TPC : (qp + 1) * TPC] = res.results[core]["out_sh"]
    return out


if __name__ == "__main__":
    import reference

    inp = reference.setup_inputs()
    inp = {k: np.asarray(v) for k, v in inp.items()}
    got = kernel(**inp)
    exp = np.asarray(reference.reference(**reference.setup_inputs()))
    err = np.abs(got - exp)
    print("abs err max:", err.max(), "rel:", err.max() / np.abs(exp).max())



# revision 4
# speedup vs baseline: 3520.6826x; 3520.6826x over previous
"""DDiT block (adaLN + masked attention + MLP) on 8 TRN2 NeuronCores.

Sharding: data-parallel over (batch, seq): core c handles tokens
[ (c%4)*1024, (c%4+1)*1024 ) of batch c//4.  K/V are AllGathered within
each batch group of 4 cores.  All matmuls run in bf16 with f32 PSUM
accumulation; layernorms, softmax statistics and residuals stay f32.
"""

import sys
from contextlib import ExitStack

sys.path.insert(0, "/opt/trn_rl_repo")

import numpy as np
import ml_dtypes

import concourse.bass as bass
import concourse.mybir as mybir
import concourse.tile as tile
import concourse.bass_utils as _bu
import concourse.bass2jax as _b2j
from concourse.bass_utils import run_bass_kernel_spmd
from concourse.masks import make_identity

# ---------------------------------------------------------------------------
# Patch: this container's walrus accepts at most ONE sync-wait per
# instruction, but Tile emits multi-wait instructions.  Split the extras
# into preceding NoOps on the same engine (engines execute their streams
# in-order, so blocking at a preceding NoOp is semantically identical).
# ---------------------------------------------------------------------------
import json as _json

_orig_compile_bir_kernel = _bu.compile_bir_kernel


def _split_multi_waits_json(bir_json):
    j = _json.loads(bir_json)
    for fn in j.get("functions", []):
        for bb in fn.get("blocks", []):
            new_insts = []
            for ins in bb.get("instructions", []):
                si = ins.get("sync_info")
                if si and si.get("on_wait") and len(si["on_wait"]) > 1:
                    waits = si["on_wait"]
                    for k, w in enumerate(waits[:-1]):
                        new_insts.append(
                            {
                                "debug": ins.get("debug"),
                                "engine": ins["engine"],
                                "ins": [],
                                "outs": [],
                                "name": f"{ins['name']}-w{k}",
                                "opcode": "NoOp",
                                "sync_info": {"on_update": [], "on_wait": [w]},
                            }
                        )
                    si["on_wait"] = [waits[-1]]
                new_insts.append(ins)
            bb["instructions"] = new_insts
    return _json.dumps(j).encode()


def _patched_compile_bir_kernel(bir_json, tmpdir, neff_name="file.neff"):
    return _orig_compile_bir_kernel(_split_multi_waits_json(bir_json), tmpdir, neff_name)


_bu.compile_bir_kernel = _patched_compile_bir_kernel
_b2j.compile_bir_kernel = _patched_compile_bir_kernel

# ---------------------------------------------------------------------------

BF = mybir.dt.bfloat16
F32 = mybir.dt.float32
AF = mybir.ActivationFunctionType
OP = mybir.AluOpType
NPBF = ml_dtypes.bfloat16

N_CORES = 8
B, S, D, H, HD = 2, 4096, 768, 12, 64
TPC = (B * S) // N_CORES  # 1024 tokens per core
NTT = TPC // 128  # 8 token tiles per core
NKT = S // 128  # 32 key tiles per batch
DC = D // 128  # 6 dim chunks
MLPD = 4 * D  # 3072
MC = MLPD // 128  # 24 mlp chunks
LN_EPS = 1e-5
AGR = 1920  # per-rank rows in the all-gather payload


def _mm(nc, out, lhsT, rhs, start, stop, tp=None):
    nc.tensor.matmul(out, lhsT, rhs, start=start, stop=stop, tile_position=tp)


def build_program():
    nc = bass.Bass("TRN2", target_bir_lowering=False, debug=False, num_devices=N_CORES)

    di = {}

    def din(name, shape, dtype):
        di[name] = nc.dram_tensor(name, shape, dtype, kind="ExternalInput")
        return di[name]

    x_sh = din("x_sh", [TPC, D], F32)
    c_col = din("c_col", [128, DC], F32)
    mq_d = din("mq", [128, NTT], F32)
    mbias_d = din("mbias", [128, NKT], F32)
    wqk_d = din("wqkT", [D, 2 * D], BF)
    wqkp_d = din("wqkTp", [D, 2 * D], BF)
    wv_d = din("wvT", [D, D], BF)
    wout_d = din("woutT", [D, D], BF)
    w1_d = din("w1T", [D, MLPD], BF)
    w2_d = din("w2T", [MLPD, D], BF)
    wadam_d = din("wadam", [D, 4 * D], BF)
    wadag_d = din("wadag", [D, 2 * D], BF)
    badam_d = din("badam", [128, 4 * DC], F32)
    badag_d = din("badag", [1, 2 * D], F32)
    b1_d = din("b1c", [128, MC], F32)
    b2_d = din("b2r", [1, D], F32)
    ln1_d = din("ln1c", [128, DC], F32)
    ln2_d = din("ln2c", [128, DC], F32)
    cosT2_d = din("cosT2", [128, TPC], BF)
    sinT2_d = din("sinT2s", [128, TPC], BF)
    cosV_d = din("cosV", [TPC, D], BF)
    sinV_d = din("sinVs", [TPC, D], BF)

    out_sh = nc.dram_tensor("out_sh", [TPC, D], F32, kind="ExternalOutput")

    def chunked(dram_ap, nchunk):
        # [nchunk*128, F] dram -> [128, nchunk, F] view for SBUF load
        return dram_ap[:].rearrange("(c p) f -> p c f", p=128)

    with tile.TileContext(nc) as tc, ExitStack() as ctx:
        const = ctx.enter_context(tc.tile_pool(name="const", bufs=1))
        # ---- constants ----
        ident = const.tile([128, 128], BF)
        make_identity(nc, ident)
        ones_col = const.tile([128, 1], BF)
        nc.vector.memset(ones_col, 1.0)
        ones_row = const.tile([1, 128], BF)
        nc.vector.memset(ones_row, 1.0)
        eps_t = const.tile([128, 1], F32)
        nc.vector.memset(eps_t, LN_EPS)
        mq_sb = const.tile([128, NTT], F32)
        nc.sync.dma_start(out=mq_sb, in_=mq_d[:])
        mq1m = const.tile([128, NTT], F32)
        nc.vector.tensor_scalar(mq1m, mq_sb, -1.0, 1.0, op0=OP.mult, op1=OP.add)
        mbias_sb = const.tile([128, NKT], F32)
        nc.sync.dma_start(out=mbias_sb, in_=mbias_d[:])
        c_sb = const.tile([128, DC], F32)
        nc.sync.dma_start(out=c_sb, in_=c_col[:])
        c_bf = const.tile([128, DC], BF)
        nc.vector.tensor_copy(c_bf, c_sb)
        cosT2 = const.tile([128, TPC], BF)
        nc.sync.dma_start(out=cosT2, in_=cosT2_d[:])
        sinT2 = const.tile([128, TPC], BF)
        nc.sync.dma_start(out=sinT2, in_=sinT2_d[:])
        ln1c = const.tile([128, DC], F32)
        nc.sync.dma_start(out=ln1c, in_=ln1_d[:])
        ln2c = const.tile([128, DC], F32)
        nc.sync.dma_start(out=ln2c, in_=ln2_d[:])
        badam_sb = const.tile([128, 4 * DC], F32)
        nc.sync.dma_start(out=badam_sb, in_=badam_d[:])
        badag_sb = const.tile([1, 2 * D], F32)
        nc.sync.dma_start(out=badag_sb, in_=badag_d[:])
        b1c_sb = const.tile([128, MC], F32)
        nc.sync.dma_start(out=b1c_sb, in_=b1_d[:])
        b2r_sb = const.tile([1, D], F32)
        nc.sync.dma_start(out=b2r_sb, in_=b2_d[:])

        modm = const.tile([128, 4 * DC], F32)  # shift_msa|scale_msa|shift_mlp|scale_mlp
        gates_row = const.tile([1, 2 * D], F32)  # gate_msa | gate_mlp
        A1 = const.tile([128, DC], F32)
        A2 = const.tile([128, DC], F32)
        B1bf = const.tile([128, DC], BF)
        B2bf = const.tile([128, DC], BF)
        gateB = const.tile([128, D], F32)
        gmlB = const.tile([128, D], F32)
        gyuB = const.tile([128, D], F32)
        gb2B = const.tile([128, D], F32)
        bq_sb = const.tile([128, 2 * DC], F32)
        bqp_sb = const.tile([128, 2 * DC], F32)
        bv_bf = const.tile([1, D], BF)

        # dram staging for the all-gather
        dpool = ctx.enter_context(tc.tile_pool(name="dram", bufs=1, space="DRAM"))
        agin = dpool.tile([AGR, 1024], BF)
        agout = dpool.tile([4 * AGR, 1024], BF)
        bc_dram = dpool.tile([4, D], F32)

        def bcast_row(out_tile, row_ap, slot):
            nc.sync.dma_start(out=bc_dram[slot : slot + 1, :], in_=row_ap)
            nc.sync.dma_start(
                out=out_tile,
                in_=bass.AP(
                    tensor=bc_dram.tensor,
                    offset=bc_dram.offset + slot * D,
                    ap=[[0, 128], [1, D]],
                ),
            )

        # ---- adaLN modulation ----
        with (
            tc.tile_pool(name="ada", bufs=1) as ada,
            tc.tile_pool(name="psA", bufs=1, space="PSUM") as psA,
        ):
            wadam_sb = ada.tile([128, DC, 4 * D], BF)
            nc.sync.dma_start(out=wadam_sb, in_=chunked(wadam_d, DC))
            wadag_sb = ada.tile([128, DC, 2 * D], BF)
            nc.sync.dma_start(out=wadag_sb, in_=chunked(wadag_d, DC))

            mod_ps = psA.tile([128, 4 * DC], F32)
            for jc in range(4 * DC):
                for ic in range(DC):
                    _mm(nc, mod_ps[:, jc : jc + 1],
                        wadam_sb[:, ic, jc * 128 : (jc + 1) * 128],
                        c_bf[:, ic : ic + 1], ic == 0, ic == DC - 1)
            nc.vector.tensor_add(modm, mod_ps, badam_sb)

            for nch in range(3):
                gps = psA.tile([1, 512], F32, tag="gps")
                for ic in range(DC):
                    _mm(nc, gps, c_bf[:, ic : ic + 1],
                        wadag_sb[:, ic, nch * 512 : (nch + 1) * 512], ic == 0, ic == DC - 1)
                nc.vector.tensor_add(
                    gates_row[:, nch * 512 : (nch + 1) * 512], gps,
                    badag_sb[:, nch * 512 : (nch + 1) * 512])

            # A1 = ln1w*(1+scale_msa); A2 = ln2w*(1+scale_mlp)
            tmp1 = ada.tile([128, DC], F32)
            nc.vector.tensor_scalar_add(tmp1, modm[:, DC : 2 * DC], 1.0)
            nc.vector.tensor_mul(A1, tmp1, ln1c)
            tmp2 = ada.tile([128, DC], F32)
            nc.vector.tensor_scalar_add(tmp2, modm[:, 3 * DC : 4 * DC], 1.0)
            nc.vector.tensor_mul(A2, tmp2, ln2c)
            nc.vector.tensor_copy(B1bf, modm[:, 0:DC])
            nc.vector.tensor_copy(B2bf, modm[:, 2 * DC : 3 * DC])

            bcast_row(gateB, gates_row[0:1, 0:D], 0)
            bcast_row(gmlB, gates_row[0:1, D : 2 * D], 1)
            r2row = ada.tile([1, D], F32)
            nc.vector.tensor_mul(r2row, gates_row[0:1, D : 2 * D], b2r_sb)
            bcast_row(gb2B, r2row, 2)

        # persistent activation pools (LIFO: x2p -> outp -> qp -> inner phases)
        x2p = ctx.enter_context(tc.tile_pool(name="x2p", bufs=1))
        x2_sb = x2p.tile([128, NTT, D], F32)
        outp_cm = tc.tile_pool(name="outp", bufs=1)
        outp = outp_cm.__enter__()
        oTall = outp.tile([128, DC, TPC], BF)
        woutT_sb = outp.tile([128, DC, D], BF)
        mv_bf = outp.tile([128, DC], BF)
        yu_row = outp.tile([1, D], F32)
        qp_cm = tc.tile_pool(name="qp", bufs=1)
        qp = qp_cm.__enter__()
        qT_sb = qp.tile([128, DC, TPC], BF)

        # ---- phase B: LN1, transposes, qkv + rope, AG staging ----
        with (
            tc.tile_pool(name="bpool", bufs=1) as bp,
            tc.tile_pool(name="wqk", bufs=1) as wq,
            tc.tile_pool(name="lnt", bufs=3) as lnt,
            tc.tile_pool(name="ropep", bufs=3) as rp,
        ):
            xhatT = bp.tile([128, DC, TPC], BF)
            kTloc = bp.tile([128, DC, TPC], BF)
            vloc = bp.tile([128, NTT, D], BF)

            wqk_sb = wq.tile([128, DC, 2 * D], BF)
            nc.sync.dma_start(out=wqk_sb, in_=chunked(wqk_d, DC))
            wqkp_sb = wq.tile([128, DC, 2 * D], BF)
            nc.sync.dma_start(out=wqkp_sb, in_=chunked(wqkp_d, DC))
            wv_sb = wq.tile([128, DC, D], BF)
            nc.sync.dma_start(out=wv_sb, in_=chunked(wv_d, DC))

            # LN1 + PE transposes
            psT_cm = tc.tile_pool(name="psT", bufs=2, space="PSUM")
            psT = psT_cm.__enter__()
            for tt in range(NTT):
                x_t = lnt.tile([128, D], F32, tag="x", bufs=2)
                nc.sync.dma_start(out=x_t, in_=x_sh[tt * 128 : (tt + 1) * 128, :])
                stats = lnt.tile([128, 3, 6], F32, tag="st")
                for sg in range(3):
                    nc.vector.bn_stats(out=stats[:, sg, :], in_=x_t[:, sg * 256 : (sg + 1) * 256])
                mv = lnt.tile([128, 2], F32, tag="mv")
                nc.vector.bn_aggr(out=mv, in_=stats)
                sq = lnt.tile([128, 1], F32, tag="sq")
                nc.scalar.activation(out=sq, in_=mv[:, 1:2], func=AF.Sqrt, bias=eps_t, scale=1.0)
                rstd = lnt.tile([128, 1], F32, tag="rs")
                nc.vector.reciprocal(out=rstd, in_=sq)
                nmr = lnt.tile([128, 1], F32, tag="nm")
                nc.vector.scalar_tensor_tensor(
                    out=nmr, in0=mv[:, 0:1], scalar=-1.0, in1=rstd, op0=OP.mult, op1=OP.mult)
                xhat_bf = lnt.tile([128, D], BF, tag="xh")
                nc.vector.tensor_scalar(xhat_bf, x_t, rstd, nmr, op0=OP.mult, op1=OP.add)
                for dc in range(DC):
                    pt = psT.tile([128, 128], BF, tag="pt")
                    nc.tensor.transpose(pt, xhat_bf[:, dc * 128 : (dc + 1) * 128], ident)
                    nc.vector.tensor_copy(
                        out=xhatT[:, dc, tt * 128 : (tt + 1) * 128], in_=pt)

            psT_cm.__exit__(None, None, None)

            # bias columns/rows from UNSCALED weights
            psBias_cm = tc.tile_pool(name="psBias", bufs=1, space="PSUM")
            psQ = psBias_cm.__enter__()
            psV = psQ
            pbq = psQ.tile([128, 2 * DC], F32, tag="pbq")
            pbqp = psQ.tile([128, 2 * DC], F32, tag="pbqp")
            for fc in range(2 * DC):
                for ic in range(DC):
                    _mm(nc, pbq[:, fc : fc + 1],
                        wqk_sb[:, ic, fc * 128 : (fc + 1) * 128],
                        B1bf[:, ic : ic + 1], ic == 0, ic == DC - 1)
                for ic in range(DC):
                    _mm(nc, pbqp[:, fc : fc + 1],
                        wqkp_sb[:, ic, fc * 128 : (fc + 1) * 128],
                        B1bf[:, ic : ic + 1], ic == 0, ic == DC - 1)
            nc.vector.tensor_copy(bq_sb, pbq)
            nc.vector.tensor_copy(bqp_sb, pbqp)
            bv_ps1 = psV.tile([1, 512], F32, tag="bv")
            bv_ps2 = psV.tile([1, 256], F32, tag="bv2")
            for ic in range(DC):
                _mm(nc, bv_ps1, B1bf[:, ic : ic + 1], wv_sb[:, ic, 0:512], ic == 0, ic == DC - 1)
                _mm(nc, bv_ps2, B1bf[:, ic : ic + 1], wv_sb[:, ic, 512:768], ic == 0, ic == DC - 1)
            nc.vector.tensor_copy(bv_bf[:, 0:512], bv_ps1)
            nc.vector.tensor_copy(bv_bf[:, 512:768], bv_ps2)

            psBias_cm.__exit__(None, None, None)

            # scale weights in place by A1 (per dim-chunk)
            for ic in range(DC):
                nc.vector.tensor_scalar_mul(wqk_sb[:, ic, :], wqk_sb[:, ic, :], A1[:, ic : ic + 1])
                nc.vector.tensor_scalar_mul(wqkp_sb[:, ic, :], wqkp_sb[:, ic, :], A1[:, ic : ic + 1])
                nc.vector.tensor_scalar_mul(wv_sb[:, ic, :], wv_sb[:, ic, :], A1[:, ic : ic + 1])

            # k first (feeds the AG), then q
            psQK_cm = tc.tile_pool(name="psQK", bufs=2, space="PSUM")
            psQ = psQK_cm.__enter__()
            for fc in list(range(DC, 2 * DC)) + list(range(DC)):
                dest = kTloc if fc >= DC else qT_sb
                fcd = fc % DC
                for q2 in range(2):
                    pa = psQ.tile([128, 512], F32, tag="pa")
                    pb = psQ.tile([128, 512], F32, tag="pb")
                    for ic in range(DC):
                        _mm(nc, pa, wqk_sb[:, ic, fc * 128 : (fc + 1) * 128],
                            xhatT[:, ic, q2 * 512 : (q2 + 1) * 512], ic == 0, ic == DC - 1)
                    for ic in range(DC):
                        _mm(nc, pb, wqkp_sb[:, ic, fc * 128 : (fc + 1) * 128],
                            xhatT[:, ic, q2 * 512 : (q2 + 1) * 512], ic == 0, ic == DC - 1)
                    t1 = rp.tile([128, 512], BF, tag="t1")
                    nc.vector.scalar_tensor_tensor(
                        out=t1, in0=pa, scalar=bq_sb[:, fc : fc + 1],
                        in1=cosT2[:, q2 * 512 : (q2 + 1) * 512], op0=OP.add, op1=OP.mult)
                    t2 = rp.tile([128, 512], BF, tag="t2")
                    nc.vector.scalar_tensor_tensor(
                        out=t2, in0=pb, scalar=bqp_sb[:, fc : fc + 1],
                        in1=sinT2[:, q2 * 512 : (q2 + 1) * 512], op0=OP.add, op1=OP.mult)
                    nc.vector.tensor_add(
                        dest[:, fcd, q2 * 512 : (q2 + 1) * 512], t1, t2)

            psQK_cm.__exit__(None, None, None)

            # v + rope (+ write AG staging per token tile)
            psVl_cm = tc.tile_pool(name="psVl", bufs=2, space="PSUM")
            psV = psVl_cm.__enter__()
            for tt in range(NTT):
                pv1 = psV.tile([128, 512], F32, tag="pv1")
                pv2 = psV.tile([128, 256], F32, tag="pv2")
                for ic in range(DC):
                    _mm(nc, pv1, xhatT[:, ic, tt * 128 : (tt + 1) * 128],
                        wv_sb[:, ic, 0:512], ic == 0, False)
                _mm(nc, pv1, ones_row, bv_bf[:, 0:512], False, True)
                for ic in range(DC):
                    _mm(nc, pv2, xhatT[:, ic, tt * 128 : (tt + 1) * 128],
                        wv_sb[:, ic, 512:768], ic == 0, False)
                _mm(nc, pv2, ones_row, bv_bf[:, 512:768], False, True)

                cv = rp.tile([128, D], BF, tag="cv", bufs=2)
                nc.sync.dma_start(out=cv, in_=cosV_d[tt * 128 : (tt + 1) * 128, :])
                sv = rp.tile([128, D], BF, tag="sv", bufs=2)
                nc.sync.dma_start(out=sv, in_=sinV_d[tt * 128 : (tt + 1) * 128, :])

                tm1 = rp.tile([128, D], BF, tag="tm1", bufs=2)
                nc.vector.tensor_mul(tm1[:, 0:512], pv1, cv[:, 0:512])
                nc.vector.tensor_mul(tm1[:, 512:768], pv2, cv[:, 512:768])
                tm2 = rp.tile([128, D], BF, tag="tm2", bufs=2)
                # half-swap within each head (8 heads in pv1, 4 in pv2)
                p1v = pv1.rearrange("p (h two j) -> p h two j", two=2, j=32)
                p2v = pv2.rearrange("p (h two j) -> p h two j", two=2, j=32)
                t2v = tm2.rearrange("p (h two j) -> p h two j", two=2, j=32)
                sv_v = sv.rearrange("p (h two j) -> p h two j", two=2, j=32)
                nc.vector.tensor_mul(t2v[:, 0:8, 0, :], p1v[:, :, 1, :], sv_v[:, 0:8, 0, :])
                nc.vector.tensor_mul(t2v[:, 0:8, 1, :], p1v[:, :, 0, :], sv_v[:, 0:8, 1, :])
                nc.vector.tensor_mul(t2v[:, 8:12, 0, :], p2v[:, :, 1, :], sv_v[:, 8:12, 0, :])
                nc.vector.tensor_mul(t2v[:, 8:12, 1, :], p2v[:, :, 0, :], sv_v[:, 8:12, 1, :])
                nc.vector.tensor_add(vloc[:, tt, :], tm1, tm2)

            # meanV partial sums over this core's tokens
            pmv = psV.tile([128, DC], F32, tag="pmv")
            for dc in range(DC):
                for tt in range(NTT):
                    _mm(nc, pmv[:, dc : dc + 1],
                        vloc[:, tt, dc * 128 : (dc + 1) * 128], ones_col,
                        tt == 0, tt == NTT - 1)
            mvp_bf = bp.tile([128, DC], BF)
            nc.vector.tensor_copy(mvp_bf, pmv)
            psVl_cm.__exit__(None, None, None)

            # stage the AG payload
            for fc in range(DC):
                nc.sync.dma_start(out=agin[fc * 128 : (fc + 1) * 128, :], in_=kTloc[:, fc, :])
            for tt in range(NTT):
                nc.sync.dma_start(
                    out=agin[D + tt * 128 : D + (tt + 1) * 128, 0:D], in_=vloc[:, tt, :])
            nc.sync.dma_start(out=agin[1792:1920, 0:DC], in_=mvp_bf)

            nc.gpsimd.collective_compute(
                "AllGather",
                OP.bypass,
                replica_groups=[[0, 1, 2, 3], [4, 5, 6, 7]],
                ins=[agin.opt()],
                outs=[agout.opt()],
            )

        # ---- attention ----
        with (
            tc.tile_pool(name="kv", bufs=1) as kv,
            tc.tile_pool(name="wp", bufs=3) as wp,
            tc.tile_pool(name="sp", bufs=1) as sp,
            tc.tile_pool(name="psSc", bufs=2, space="PSUM") as psSc,
            tc.tile_pool(name="psO", bufs=1, space="PSUM") as psO,
            tc.tile_pool(name="psS", bufs=1, space="PSUM") as psS,
        ):
            KTf = kv.tile([128, DC, S], BF)
            Vf = kv.tile([128, NKT, D], BF)
            for r in range(4):
                for fc in range(DC):
                    nc.sync.dma_start(
                        out=KTf[:, fc, r * 1024 : (r + 1) * 1024],
                        in_=agout[r * AGR + fc * 128 : r * AGR + (fc + 1) * 128, :])
                for tt in range(NTT):
                    nc.sync.dma_start(
                        out=Vf[:, r * NTT + tt, :],
                        in_=agout[r * AGR + D + tt * 128 : r * AGR + D + (tt + 1) * 128, 0:D])
            nc.sync.dma_start(out=woutT_sb, in_=chunked(wout_d, DC))

            # meanV: sum of 4 rank partials, then /S
            mvps = [kv.tile([128, DC], BF, tag=f"mvp{r}", name=f"mvp{r}") for r in range(4)]
            for r in range(4):
                nc.sync.dma_start(
                    out=mvps[r], in_=agout[r * AGR + 1792 : r * AGR + 1920, 0:DC])
            mvf1 = kv.tile([128, DC], F32)
            nc.vector.tensor_add(mvf1, mvps[0], mvps[1])
            mvf2 = kv.tile([128, DC], F32)
            nc.vector.tensor_add(mvf2, mvps[2], mvps[3])
            mvf = kv.tile([128, DC], F32)
            nc.vector.tensor_add(mvf, mvf1, mvf2)
            nc.vector.tensor_scalar_mul(mv_bf, mvf, 1.0 / S)

            # y_unif = Wout @ meanV   (row form), then gyuB broadcast
            yps1 = psS.tile([1, 512], F32, tag="yu1")
            yps2 = psS.tile([1, 256], F32, tag="yu2")
            for ic in range(DC):
                _mm(nc, yps1, mv_bf[:, ic : ic + 1], woutT_sb[:, ic, 0:512], ic == 0, ic == DC - 1)
                _mm(nc, yps2, mv_bf[:, ic : ic + 1], woutT_sb[:, ic, 512:768], ic == 0, ic == DC - 1)
            nc.vector.tensor_copy(yu_row[:, 0:512], yps1)
            nc.vector.tensor_copy(yu_row[:, 512:768], yps2)
            gyu_row = kv.tile([1, D], F32)
            nc.vector.tensor_mul(gyu_row, yu_row, gates_row[0:1, 0:D])
            bcast_row(gyuB, gyu_row, 3)

            rs_dram = dpool.tile([2, 512], F32)

            for hp in range(DC):
                for q2 in range(2):
                    oT_ps = psO.tile([128, 512], F32, tag="ot")
                    s_ps = psS.tile([128, 512], F32, tag="sps")
                    for kt in range(NKT):
                        sc = psSc.tile([128, 1024], F32, tag="sc")
                        _mm(nc, sc[:, 0:512],
                            KTf[0:64, hp, kt * 128 : (kt + 1) * 128],
                            qT_sb[0:64, hp, q2 * 512 : (q2 + 1) * 512],
                            True, True, tp=(0, 0))
                        _mm(nc, sc[:, 512:1024],
                            KTf[64:128, hp, kt * 128 : (kt + 1) * 128],
                            qT_sb[64:128, hp, q2 * 512 : (q2 + 1) * 512],
                            True, True, tp=(64, 0))
                        w = wp.tile([128, 1024], BF, tag="w")
                        nc.scalar.activation(
                            out=w, in_=sc, func=AF.Exp,
                            bias=mbias_sb[:, kt : kt + 1], scale=0.125)
                        _mm(nc, oT_ps[0:64, :],
                            Vf[:, kt, hp * 128 : hp * 128 + 64], w[:, 0:512],
                            kt == 0, kt == NKT - 1, tp=(0, 0))
                        _mm(nc, oT_ps[64:128, :],
                            Vf[:, kt, hp * 128 + 64 : hp * 128 + 128], w[:, 512:1024],
                            kt == 0, kt == NKT - 1, tp=(0, 64))
                        _mm(nc, s_ps[0:1, :], ones_col, w[:, 0:512],
                            kt == 0, kt == NKT - 1, tp=(0, 0))
                        _mm(nc, s_ps[32:33, :], ones_col, w[:, 512:1024],
                            kt == 0, kt == NKT - 1, tp=(0, 32))
                    rsA = sp.tile([1, 512], F32, tag="rsA")
                    nc.vector.reciprocal(out=rsA, in_=s_ps[0:1, :])
                    rsB = sp.tile([1, 512], F32, tag="rsB")
                    nc.vector.reciprocal(out=rsB, in_=s_ps[32:33, :])
                    # broadcast via dram bounce (partition-step-0 read)
                    nc.sync.dma_start(out=rs_dram[0:1, :], in_=rsA)
                    nc.sync.dma_start(out=rs_dram[1:2, :], in_=rsB)
                    rsb = sp.tile([128, 512], F32, tag="rsb")
                    nc.sync.dma_start(
                        out=rsb[0:64, :],
                        in_=bass.AP(tensor=rs_dram.tensor, offset=rs_dram.offset,
                                    ap=[[0, 64], [1, 512]]))
                    nc.sync.dma_start(
                        out=rsb[64:128, :],
                        in_=bass.AP(tensor=rs_dram.tensor, offset=rs_dram.offset + 512,
                                    ap=[[0, 64], [1, 512]]))
                    nc.vector.tensor_mul(
                        oTall[:, hp, q2 * 512 : (q2 + 1) * 512], oT_ps, rsb)

        # ---- out-proj + residual/blend ----
        with (
            tc.tile_pool(name="xop", bufs=3) as xop,
            tc.tile_pool(name="psY", bufs=2, space="PSUM") as psY,
        ):
            for tt in range(NTT):
                y1 = psY.tile([128, 512], F32, tag="y1")
                y2 = psY.tile([128, 256], F32, tag="y2")
                for fc in range(DC):
                    _mm(nc, y1, oTall[:, fc, tt * 128 : (tt + 1) * 128],
                        woutT_sb[:, fc, 0:512], fc == 0, fc == DC - 1)
                for fc in range(DC):
                    _mm(nc, y2, oTall[:, fc, tt * 128 : (tt + 1) * 128],
                        woutT_sb[:, fc, 512:768], fc == 0, fc == DC - 1)
                xs = xop.tile([128, D], F32, tag="xs")
                nc.sync.dma_start(out=xs, in_=x_sh[tt * 128 : (tt + 1) * 128, :])
                t1 = xop.tile([128, D], F32, tag="t1")
                nc.vector.scalar_tensor_tensor(
                    out=t1[:, 0:512], in0=y1, scalar=mq_sb[:, tt : tt + 1],
                    in1=gateB[:, 0:512], op0=OP.mult, op1=OP.mult)
                nc.vector.scalar_tensor_tensor(
                    out=t1[:, 512:768], in0=y2, scalar=mq_sb[:, tt : tt + 1],
                    in1=gateB[:, 512:768], op0=OP.mult, op1=OP.mult)
                t2 = xop.tile([128, D], F32, tag="t2")
                nc.vector.scalar_tensor_tensor(
                    out=t2, in0=gyuB, scalar=mq1m[:, tt : tt + 1], in1=xs,
                    op0=OP.mult, op1=OP.add)
                nc.vector.tensor_add(x2_sb[:, tt, :], t1, t2)

        qp_cm.__exit__(None, None, None)
        outp_cm.__exit__(None, None, None)

        # ---- LN2 + MLP ----
        with (
            tc.tile_pool(name="mlp", bufs=1) as mp,
            tc.tile_pool(name="ln2t", bufs=3) as ln2t,
            tc.tile_pool(name="psT2", bufs=1, space="PSUM") as psT2,
            tc.tile_pool(name="psH", bufs=2, space="PSUM") as psH,
            tc.tile_pool(name="psM", bufs=1, space="PSUM") as psM,
        ):
            xhat2T = mp.tile([128, DC, TPC], BF)
            for tt in range(NTT):
                stats = ln2t.tile([128, 3, 6], F32, tag="st")
                for sg in range(3):
                    nc.vector.bn_stats(
                        out=stats[:, sg, :], in_=x2_sb[:, tt, sg * 256 : (sg + 1) * 256])
                mv = ln2t.tile([128, 2], F32, tag="mv")
                nc.vector.bn_aggr(out=mv, in_=stats)
                sq = ln2t.tile([128, 1], F32, tag="sq")
                nc.scalar.activation(out=sq, in_=mv[:, 1:2], func=AF.Sqrt, bias=eps_t, scale=1.0)
                rstd = ln2t.tile([128, 1], F32, tag="rs")
                nc.vector.reciprocal(out=rstd, in_=sq)
                nmr = ln2t.tile([128, 1], F32, tag="nm")
                nc.vector.scalar_tensor_tensor(
                    out=nmr, in0=mv[:, 0:1], scalar=-1.0, in1=rstd, op0=OP.mult, op1=OP.mult)
                xh2 = ln2t.tile([128, D], BF, tag="xh", bufs=2)
                nc.vector.tensor_scalar(xh2, x2_sb[:, tt, :], rstd, nmr, op0=OP.mult, op1=OP.add)
                for dc in range(DC):
                    pt = psT2.tile([128, 128], BF, tag="pt")
                    nc.tensor.transpose(pt, xh2[:, dc * 128 : (dc + 1) * 128], ident)
                    nc.vector.tensor_copy(
                        out=xhat2T[:, dc, tt * 128 : (tt + 1) * 128], in_=pt)

            w1_sb = mp.tile([128, DC, MLPD], BF)
            nc.sync.dma_start(out=w1_sb, in_=chunked(w1_d, DC))
            # b1' from unscaled W1
            pb1 = psH.tile([128, MC], F32, tag="pb1")
            for fc1 in range(MC):
                for ic in range(DC):
                    _mm(nc, pb1[:, fc1 : fc1 + 1],
                        w1_sb[:, ic, fc1 * 128 : (fc1 + 1) * 128],
                        B2bf[:, ic : ic + 1], ic == 0, ic == DC - 1)
            b1full = mp.tile([128, MC], F32)
            nc.vector.tensor_add(b1full, pb1, b1c_sb)
            for ic in range(DC):
                nc.vector.tensor_scalar_mul(w1_sb[:, ic, :], w1_sb[:, ic, :], A2[:, ic : ic + 1])

            g1_sb = mp.tile([128, MC, TPC], BF)
            for fc1 in range(MC):
                for q2 in range(2):
                    hps = psH.tile([128, 512], F32, tag="hps")
                    for ic in range(DC):
                        _mm(nc, hps, w1_sb[:, ic, fc1 * 128 : (fc1 + 1) * 128],
                            xhat2T[:, ic, q2 * 512 : (q2 + 1) * 512], ic == 0, ic == DC - 1)
                    nc.scalar.activation(
                        out=g1_sb[:, fc1, q2 * 512 : (q2 + 1) * 512], in_=hps,
                        func=AF.Gelu_apprx_tanh, bias=b1full[:, fc1 : fc1 + 1], scale=1.0)

            w2_sb = mp.tile([128, MC, D], BF)
            nc.sync.dma_start(out=w2_sb, in_=chunked(w2_d, MC))
            for tt in range(NTT):
                m1 = psM.tile([128, 512], F32, tag="m1")
                m2 = psM.tile([128, 256], F32, tag="m2")
                for fc1 in range(MC):
                    _mm(nc, m1, g1_sb[:, fc1, tt * 128 : (tt + 1) * 128],
                        w2_sb[:, fc1, 0:512], fc1 == 0, fc1 == MC - 1)
                for fc1 in range(MC):
                    _mm(nc, m2, g1_sb[:, fc1, tt * 128 : (tt + 1) * 128],
                        w2_sb[:, fc1, 512:768], fc1 == 0, fc1 == MC - 1)
                u1 = ln2t.tile([128, D], F32, tag="u1", bufs=2)
                nc.vector.tensor_mul(u1[:, 0:512], m1, gmlB[:, 0:512])
                nc.vector.tensor_mul(u1[:, 512:768], m2, gmlB[:, 512:768])
                x3 = ln2t.tile([128, D], F32, tag="x3", bufs=2)
                nc.vector.tensor_add(x3, u1, x2_sb[:, tt, :])
                nc.vector.tensor_add(x3, x3, gb2B)
                nc.sync.dma_start(out=out_sh[tt * 128 : (tt + 1) * 128, :], in_=x3)

    return nc


_PROGRAM = None
_LAST_IN_MAPS = None


def _get_program():
    global _PROGRAM
    if _PROGRAM is None:
        _PROGRAM = build_program()
    return _PROGRAM


def _bf(a):
    return np.ascontiguousarray(np.asarray(a, np.float32)).astype(NPBF)


def kernel(x, cos, sin, c, attention_mask, ln1_w, Wqkv, Wout, ln2_w, W1, b1, W2, b2,
           Wada, bada):
    x = np.asarray(x, np.float32)
    cos2d = np.asarray(cos, np.float32).reshape(S, HD)
    sin2d = np.asarray(sin, np.float32).reshape(S, HD)
    c = np.asarray(c, np.float32)
    m = np.asarray(attention_mask)
    mf = m.astype(np.float32)

    WqkvT = np.asarray(Wqkv, np.float32).T  # [768, 2304]
    f = np.arange(2 * D)
    perm = (f // HD) * HD + (f % HD + HD // 2) % HD
    wqkT = _bf(WqkvT[:, : 2 * D])
    wqkTp = _bf(WqkvT[:, : 2 * D][:, perm])
    wvT = _bf(WqkvT[:, 2 * D :])
    woutT = _bf(np.asarray(Wout, np.float32).T)
    w1T = _bf(np.asarray(W1, np.float32).T)
    w2T = _bf(np.asarray(W2, np.float32).T)
    WadaT = np.asarray(Wada, np.float32).T  # [768, 4608]
    wadam = _bf(np.concatenate(
        [WadaT[:, 0:768], WadaT[:, 768:1536], WadaT[:, 2304:3072], WadaT[:, 3072:3840]],
        axis=1))
    wadag = _bf(np.concatenate([WadaT[:, 1536:2304], WadaT[:, 3840:4608]], axis=1))
    bada = np.asarray(bada, np.float32)
    badam = np.ascontiguousarray(
        np.concatenate([bada[0:768], bada[768:1536], bada[2304:3072], bada[3072:3840]])
        .reshape(4 * DC, 128).T)
    badag = np.ascontiguousarray(
        np.concatenate([bada[1536:2304], bada[3840:4608]])[None, :])
    b1c = np.ascontiguousarray(np.asarray(b1, np.float32).reshape(MC, 128).T)
    b2r = np.ascontiguousarray(np.asarray(b2, np.float32)[None, :])
    ln1c = np.ascontiguousarray(np.asarray(ln1_w, np.float32).reshape(DC, 128).T)
    ln2c = np.ascontiguousarray(np.asarray(ln2_w, np.float32).reshape(DC, 128).T)

    sgn = np.where(np.arange(HD) < HD // 2, -1.0, 1.0).astype(np.float32)
    cosT = cos2d.T  # [64, 4096]
    sinTs = sin2d.T * sgn[:, None]
    cosT2_full = np.tile(cosT, (2, 1))  # [128, 4096]
    sinT2_full = np.tile(sinTs, (2, 1))
    cosV_full = np.tile(cos2d, (1, H))  # [4096, 768]
    sinV_full = np.tile(sin2d * sgn[None, :], (1, H))

    nc = _get_program()

    in_maps = []
    for core in range(N_CORES):
        bi, qp = core // 4, core % 4
        t0, t1 = qp * TPC, (qp + 1) * TPC
        mb_core = np.ascontiguousarray(
            ((mf[bi] - 1.0) * 1e9).reshape(NKT, 128).T)  # [128, 32]
        mq_core = np.ascontiguousarray(mf[bi, t0:t1].reshape(NTT, 128).T)  # [128, 8]
        c_core = np.ascontiguousarray(c[bi].reshape(DC, 128).T)  # [128, 6]
        in_maps.append({
            "x_sh": np.ascontiguousarray(x[bi, t0:t1]),
            "c_col": c_core,
            "mq": mq_core,
            "mbias": mb_core,
            "wqkT": wqkT, "wqkTp": wqkTp, "wvT": wvT, "woutT": woutT,
            "w1T": w1T, "w2T": w2T, "wadam": wadam, "wadag": wadag,
            "badam": badam, "badag": badag, "b1c": b1c, "b2r": b2r,
            "ln1c": ln1c, "ln2c": ln2c,
            "cosT2": _bf(cosT2_full[:, t0:t1]),
            "sinT2s": _bf(sinT2_full[:, t0:t1]),
            "cosV": _bf(cosV_full[t0:t1]),
            "sinVs": _bf(sinV_full[t0:t1]),
        })

    global _LAST_IN_MAPS
    _LAST_IN_MAPS = in_maps
    res = run_bass_kernel_spmd(nc, in_maps, core_ids=list(range(N_CORES)))

    out = np.empty((B, S, D), np.float32)
    for core in range(N_CORES):
        bi, qp = core // 4, core % 4
        out[bi, qp * TPC : (qp + 1) * TPC] = res.results[core]["out_sh"]
    return out


if __name__ == "__main__":
    import reference

    inp = reference.setup_inputs()
    inp = {k: np.asarray(v) for k, v in inp.items()}
    got = kernel(**inp)
    exp = np.asarray(reference.reference(**reference.setup_inputs()))
    err = np.abs(got - exp)
    print("abs err max:", err.max(), "rel:", err.max() / np.abs(exp).max())

